# revision 40
# baseline (speedup 1.0000x reference)
"""DynamicSincConv1d Trainium2 kernel (v2).

Data-parallel over batch: 8 batch elements -> 8 NeuronCores. All heavy
math runs on-device; the host reshapes inputs into DMA-friendly layouts
and reassembles the output.

Key algorithmic moves vs a direct translation:
 - The windowed-sinc bank is symmetric about k=128, so every filter
   spectrum is real: the complex multiply becomes a real scaling of X
   (fbpack layout: pack a = Re fb 0..127, pack b = [fb128, Im fb 1..127]).
 - sinc(w*t) is approximated by a degree-3 polynomial in (w*t)^2
   (max err 2e-6 over the reachable |w|<=1, |t|<=0.5 range).  The
   filter synthesis + rFFT then factorizes through 4 moments
   P_j = amp*w^(2j): R = WS @ Pm with host-baked WS.  This removes the
   per-(o,i,s) sinc/sin/reciprocal elementwise work and two matmul
   layers entirely.
 - Stage-1 conv uses m=128 output tiles (4 kernel-shifts x 32 channels)
   and a cheap cross-partition j-sum, 4x fewer matmul rows.
 - iSTFT overlap-add folds into PSUM accumulation with shifted rhs
   slices; the window-square normalization is periodic except at the
   two boundary hops, so it reduces to a per-partition tensor_scalar.
 - Matmuls run in fp16/fp32r (1 cycle/row vs 4 for fp32); elementwise
   stages run in fp16 (2x DVE throughput).
"""

import math
import numpy as np

B, CI, I, O, S = 8, 32, 2, 4, 4
K, HOP, T = 256, 64, 65536
F = T // HOP + 1            # 1025 frames
H = (T + K) // HOP          # 1028
PI = math.pi
FT_F = [(0, 512), (512, 512), (1024, 1)]     # tiles of F
FT_H = [(0, 512), (512, 512), (1024, 4)]     # tiles of H

_prog_cache = {}


def _consts():
    n = np.arange(K, dtype=np.float64)
    ola = 0.5 * (1.0 - np.cos(2.0 * PI * n / K))
    fir = 0.42 - 0.5 * np.cos(2.0 * PI * n / K) + 0.08 * np.cos(4.0 * PI * n / K)

    d = np.arange(128, dtype=np.float64)
    fb = np.arange(K // 2 + 1, dtype=np.float64)
    cd = np.where(d == 0, 1.0, 2.0)
    m1full = (((-1.0) ** fb)[None, :] * cd[:, None] * fir[128 + d.astype(int)][:, None]
              / (S * K) * np.cos(2.0 * PI * np.outer(d, fb) / K))     # [128, 129]

    # sinc_n(u) ~= sum_j c_j u^(2j) on u in [0, 0.503]
    us = np.linspace(0, 0.503, 6001)
    V = np.vander(us * us, 4, increasing=True)
    c, *_ = np.linalg.lstsq(V, np.sinc(us), rcond=None)

    pw = (d[:, None] / K) ** (2 * np.arange(4)[None, :])              # [128, 4]
    M2full = np.einsum("df,dj->fj", m1full, pw) * c[None, :]          # [129, 4]
    M2a = M2full[0:128]
    M2b = np.concatenate([M2full[128:129], M2full[1:128]], axis=0)    # [128, 4]

    WS = np.zeros((128, 16, 128), dtype=np.float64)
    for oi in range(8):
        for j in range(4):
            for s in range(4):
                WS[32 * j + 4 * oi + s, oi * 2 + 0, :] = M2a[:, j]
                WS[32 * j + 4 * oi + s, oi * 2 + 1, :] = M2b[:, j]
    WS = np.ascontiguousarray(WS.reshape(128, 2048)).astype(np.float16)

    kk = np.arange(K, dtype=np.float64)
    ang = 2.0 * PI * np.outer(kk, fb) / K
    wre = ola[:, None] * np.cos(ang)
    wim = -ola[:, None] * np.sin(ang)
    colsA = wre[:, 0:128]
    colsB = np.concatenate([wre[:, 128:129], wim[:, 1:128]], axis=1)
    wx_full = np.concatenate([colsA, colsB], axis=1).reshape(4, 64, 256)
    wx_h = np.ascontiguousarray(
        wx_full.transpose(1, 0, 2).reshape(64, 1024)).astype(np.float16)

    cp = np.where(fb == 0, 1.0, 2.0)
    icre = (cp[:, None] / K) * np.cos(2.0 * PI * np.outer(fb, n) / K) * ola[None, :]
    icim = (-2.0 / K) * np.sin(2.0 * PI * np.outer(fb, n) / K) * ola[None, :]
    ica = np.ascontiguousarray(icre[0:128]).astype(np.float16)        # [128, 256]
    icb = np.concatenate([icre[128:129], icim[1:128]], axis=0).astype(np.float16)

    ola2 = ola * ola
    r = np.arange(64)
    env_int = sum(ola2[64 * j + r] for j in range(4))
    env_p0 = sum(ola2[64 * j + r] for j in (0, 1, 2))
    env_p1023 = sum(ola2[64 * j + r] for j in (1, 2, 3))
    inv3 = np.stack([1.0 / env_int, 1.0 / env_p0, 1.0 / env_p1023],
                    axis=1).astype(np.float32)                        # [64, 3]

    return dict(WS=WS, wx_h=wx_h, ica=ica, icb=icb, inv3=inv3)


def _build_program():
    import concourse.bacc as bacc
    import concourse.mybir as mybir
    import concourse.tile as tile

    f32 = mybir.dt.float32
    f16 = mybir.dt.float16
    f32r = mybir.dt.float32r
    AF = mybir.ActivationFunctionType
    ALU = mybir.AluOpType

    nc = bacc.Bacc("TRN2", target_bir_lowering=False, debug=False, num_devices=8)

    eye_in = nc.dram_tensor("eye_in", [128, 128], f16, kind="ExternalInput")
    d_in = nc.dram_tensor("d_in", [16, 128, H], f16, kind="ExternalInput")
    xd_in = nc.dram_tensor("xd_in", [64, 2 * H], f16, kind="ExternalInput")
    w1n_in = nc.dram_tensor("w1n_in", [128, 2048], f16, kind="ExternalInput")
    w2r_in = nc.dram_tensor("w2r_in", [32, 256], f16, kind="ExternalInput")
    ws_in = nc.dram_tensor("ws_in", [128, 2048], f16, kind="ExternalInput")
    wx_in = nc.dram_tensor("wx_in", [64, 1024], f16, kind="ExternalInput")
    ica_in = nc.dram_tensor("ica_in", [128, 256], f16, kind="ExternalInput")
    icb_in = nc.dram_tensor("icb_in", [128, 256], f16, kind="ExternalInput")
    b1_in = nc.dram_tensor("b1_in", [32, 1], f32, kind="ExternalInput")
    b1s_in = nc.dram_tensor("b1s_in", [32, 1], f32, kind="ExternalInput")
    b2a_in = nc.dram_tensor("b2a_in", [128, 1], f32, kind="ExternalInput")
    b2w_in = nc.dram_tensor("b2w_in", [128, 1], f32, kind="ExternalInput")
    mk_in = nc.dram_tensor("mk_in", [128, 4], f32, kind="ExternalInput")
    inv3_in = nc.dram_tensor("inv3_in", [64, 3], f32, kind="ExternalInput")
    bias_in = nc.dram_tensor("bias_in", [64, 4], f32, kind="ExternalInput")
    yt_out = nc.dram_tensor("yt_out", [64, 4096], f32, kind="ExternalOutput")

    with tile.TileContext(nc) as tc:
        with tc.tile_pool(name="cpool", bufs=1) as cpool:
            w1n_sb = cpool.tile([128, 2048], f16, tag="w1n")
            w2r_sb = cpool.tile([32, 256], f16, tag="w2r")
            ws_sb = cpool.tile([128, 2048], f16, tag="ws")
            wx_sb = cpool.tile([64, 1024], f16, tag="wx")
            ica_sb = cpool.tile([128, 256], f16, tag="ica")
            icb_sb = cpool.tile([128, 256], f16, tag="icb")
            b1_sb = cpool.tile([32, 1], f32, tag="b1")
            b1s_sb = cpool.tile([32, 1], f32, tag="b1s")
            b2a_sb = cpool.tile([128, 1], f32, tag="b2a")
            b2w_sb = cpool.tile([128, 1], f32, tag="b2w")
            mk_sb = cpool.tile([128, 4], f32, tag="mk")
            inv3_sb = cpool.tile([64, 3], f32, tag="inv3")
            bias_sb = cpool.tile([64, 4], f32, tag="bias")
            xd_sb = cpool.tile([64, 2 * H], f16, tag="xd")
            xa_sb = cpool.tile([128, 2 * F], f16, tag="xa")
            xb_sb = cpool.tile([128, 2 * F], f16, tag="xb")
            eye_sb = cpool.tile([128, 128], f16, tag="eye")
            wid4_sb = cpool.tile([128, F], f16, tag="wid4")
            wsq4_sb = cpool.tile([128, F], f16, tag="wsq4")
            mb1_sb = cpool.tile([128, F], f16, tag="mb1")
            mb2_sb = cpool.tile([128, F], f16, tag="mb2")
            b2sq_sb = cpool.tile([128, F], f16, tag="b2sq")
            tm_sb = cpool.tile([128, F], f16, tag="tm")
            pm_sb = cpool.tile([128, F], f16, tag="pm")
            outs_sb = cpool.tile([128, H], f16, tag="outs")
            hb_sb = cpool.tile([32, F], f32, tag="hb")
            lt_sb = cpool.tile([32, F], f32, tag="lt")
            h_sb = cpool.tile([32, F], f16, tag="h")
            fpa_sb = cpool.tile([128, 8 * F], f16, tag="fpa")
            fpb_sb = cpool.tile([128, 8 * F], f16, tag="fpb")
            yt_sb = cpool.tile([64, 4096], f32, tag="yt")

            # DMA spread over the two HW DGE queues (sync + scalar).
            # STFT inputs first (gate PE start), then weights, then the
            # big conditioning stream interleaved with small consts.
            with tc.tile_pool(name="dpool", bufs=1) as dpool:
                dts = [dpool.tile([128, H], f16, tag=f"d{c}", name=f"d{c}")
                       for c in range(16)]
                nc.sync.dma_start(wx_sb[:], wx_in[:])
                nc.sync.dma_start(xd_sb[:, 0:H], xd_in[:, 0:H])
                nc.scalar.dma_start(xd_sb[:, H:2 * H], xd_in[:, H:2 * H])
                nc.scalar.dma_start(w1n_sb[:], w1n_in[:])
                nc.scalar.dma_start(ws_sb[:], ws_in[:])
                nc.scalar.dma_start(eye_sb[:], eye_in[:])
                sync_small = [(b1_sb, b1_in), (b1s_sb, b1s_in),
                              (w2r_sb, w2r_in), (b2a_sb, b2a_in),
                              (b2w_sb, b2w_in), (mk_sb, mk_in),
                              (inv3_sb, inv3_in), (bias_sb, bias_in),
                              (ica_sb, ica_in), (icb_sb, icb_in)]
                for c in range(16):
                    eng = nc.sync if c % 2 == 0 else nc.scalar
                    eng.dma_start(dts[c][:], d_in[c])
                    if c % 2 == 0 and sync_small:
                        t_sb, t_in = sync_small.pop(0)
                        nc.sync.dma_start(t_sb[:], t_in[:])
                for t_sb, t_in in sync_small:
                    nc.sync.dma_start(t_sb[:], t_in[:])

                def mm(out, lhsT, rhs, start, stop, reuse=False):
                    """matmul; reuse=True elides the stationary-weight
                    reload (previous matmul loaded the same lhsT)."""
                    inst = nc.tensor.matmul(out, lhsT, rhs,
                                            start=start, stop=stop)
                    if reuse:
                        inst.ins.ldweights = False
                    return inst

                # STFT quarters double as PE filler between dependent stages
                with tc.tile_pool(name="ps4", bufs=1, space="PSUM") as ps4:
                    def stft_quarter(i, mt, xdst):
                        ps = ps4.tile([128, F], f32, tag="ps4", name=f"ps4_{i}{mt}")
                        for j in range(4):
                            for ft, (f0, nf) in enumerate(FT_F):
                                mm(ps[:, f0:f0 + nf],
                                   wx_sb[:, j * 256 + mt * 128:
                                         j * 256 + mt * 128 + 128],
                                   xd_sb[:, i * H + f0 + j:i * H + f0 + j + nf],
                                   start=(j == 0), stop=(j == 3),
                                   reuse=(ft > 0))
                        nc.scalar.activation(xdst[:, i * F:(i + 1) * F],
                                             ps[:], AF.Copy)

                    stft_quarter(0, 0, xa_sb)

                    # ---- stage 1: conditioning conv, m=128 (4j x 32ch) ----
                    with tc.tile_pool(name="ps1", bufs=1, space="PSUM") as ps1:
                        out_ps = ps1.tile([128, H], f32, tag="out1")
                        for c in range(16):
                            for ft, (f0, nf) in enumerate(FT_H):
                                mm(out_ps[:, f0:f0 + nf],
                                   w1n_sb[:, c * 128:(c + 1) * 128],
                                   dts[c][:, f0:f0 + nf],
                                   start=(c == 0), stop=(c == 15),
                                   reuse=(ft > 0))
                        nc.scalar.activation(outs_sb[:, 0:516],
                                             out_ps[:, 0:516], AF.Copy)
                        nc.scalar.activation(outs_sb[:, 516:H],
                                             out_ps[:, 516:H], AF.Copy)

                    stft_quarter(0, 1, xb_sb)

                    # j-sum via 4 shifted accumulating matmuls (identity
                    # lhsT), then bias + leaky_relu
                    with tc.tile_pool(name="psh", bufs=1, space="PSUM") as psh:
                        h_ps = psh.tile([32, F], f32, tag="hps")
                        for j in range(4):
                            for ft, (f0, nf) in enumerate(FT_F):
                                mm(h_ps[:, f0:f0 + nf],
                                   eye_sb[:, j * 32:(j + 1) * 32],
                                   outs_sb[:, f0 + j:f0 + j + nf],
                                   start=(j == 0), stop=(j == 3),
                                   reuse=(ft > 0))
                        nc.scalar.activation(hb_sb[:], h_ps[:], AF.Identity,
                                             bias=b1_sb[:, 0:1])
                        nc.scalar.activation(lt_sb[:], h_ps[:], AF.Identity,
                                             bias=b1s_sb[:, 0:1], scale=0.01)
                        nc.vector.tensor_max(h_sb[:], hb_sb[:], lt_sb[:])

                    stft_quarter(1, 0, xa_sb)

                    # ---- stage 2: 1x1 conv (outputs replicated 4x over
                    # partition blocks) + tanh -> amp4 (in pm) / wid4 ----
                    with tc.tile_pool(name="ps2", bufs=2, space="PSUM") as ps2:
                        for (k0, dst, bcol) in ((0, pm_sb, b2a_sb),
                                                (128, wid4_sb, b2w_sb)):
                            for ft, (f0, nf) in enumerate(FT_F):
                                pa = ps2.tile([128, nf], f32, tag="ps2a")
                                mm(pa[:], w2r_sb[:, k0:k0 + 128],
                                   h_sb[:, f0:f0 + nf],
                                   start=True, stop=True, reuse=(ft > 0))
                                nc.scalar.activation(dst[:, f0:f0 + nf], pa[:],
                                                     AF.Tanh, bias=bcol[:, 0:1])

                    # ---- moments in place: pm = amp * [1,w2,w4,w6] by
                    # partition block via per-partition mask scalars:
                    # mb1 = [1,w2,1,w2], mb2 = [1,1,w2,w2], pm = amp*mb1*mb2^2
                    nc.vector.tensor_mul(wsq4_sb[:], wid4_sb[:], wid4_sb[:])
                    nc.vector.tensor_scalar(mb1_sb[:], wsq4_sb[:],
                                            mk_sb[:, 0:1], mk_sb[:, 1:2],
                                            ALU.mult, ALU.add)
                    nc.vector.tensor_scalar(mb2_sb[:], wsq4_sb[:],
                                            mk_sb[:, 2:3], mk_sb[:, 3:4],
                                            ALU.mult, ALU.add)
                    nc.gpsimd.tensor_mul(b2sq_sb[:], mb2_sb[:], mb2_sb[:])
                    nc.vector.tensor_mul(tm_sb[:], pm_sb[:], mb1_sb[:])
                    nc.vector.tensor_mul(pm_sb[:], tm_sb[:], b2sq_sb[:])

                    stft_quarter(1, 1, xb_sb)

            # ---- stage 3: R = WS @ Pm per (oi, pack) -> fpa/fpb f16 ----
            with tc.tile_pool(name="ps3", bufs=2, space="PSUM") as ps3:
                for oi in range(8):
                    for (p, dest) in ((0, fpa_sb), (1, fpb_sb)):
                        ps = ps3.tile([128, F], f32, tag="ps3")
                        for ft, (f0, nf) in enumerate(FT_F):
                            mm(ps[:, f0:f0 + nf],
                               ws_sb[:, (oi * 2 + p) * 128:
                                     (oi * 2 + p + 1) * 128],
                               pm_sb[:, f0:f0 + nf],
                               start=True, stop=True, reuse=(ft > 0))
                        dst = dest[:, oi * F:(oi + 1) * F]
                        if p == 0 or oi % 2 == 0:
                            nc.scalar.activation(dst, ps[:], AF.Copy)
                        else:
                            nc.vector.tensor_scalar(dst, ps[:], 1.0,
                                                    None, ALU.mult)

            # ---- stage 5+6 per o: cmul, iSTFT with OLA in PSUM ----
            with tc.tile_pool(name="yp", bufs=2) as ypool, \
                 tc.tile_pool(name="ctp", bufs=2) as ctpool, \
                 tc.tile_pool(name="ps6", bufs=4, space="PSUM") as ps6:
                for o in range(4):
                    ya = ypool.tile([128, H], f16, tag="ya")
                    yb = ypool.tile([128, H], f16, tag="yb")
                    ta_t = ctpool.tile([128, 2 * F], f16, tag="cta")
                    tb_t = ctpool.tile([128, 2 * F], f16, tag="ctb")
                    o2 = 2 * o
                    nc.gpsimd.memset(ya[:, 0:1], 0.0)
                    nc.gpsimd.memset(ya[:, 1026:1028], 0.0)
                    nc.gpsimd.memset(yb[:, 0:1], 0.0)
                    nc.gpsimd.memset(yb[:, 1026:1028], 0.0)
                    # all on DVE (gpsimd is ~2-3x slower on wide ops); adds
                    # split at col 515 so the pt=0 iSTFT matmuls can start
                    # before the back half of ya/yb is summed
                    nc.vector.tensor_mul(ta_t[:], xa_sb[:],
                                         fpa_sb[:, o2 * F:(o2 + 2) * F])
                    nc.vector.tensor_mul(tb_t[:], xb_sb[:],
                                         fpb_sb[:, o2 * F:(o2 + 2) * F])
                    nc.vector.tensor_add(ya[:, 1:516], ta_t[:, 0:515],
                                         ta_t[:, F:F + 515])
                    nc.vector.tensor_add(yb[:, 1:516], tb_t[:, 0:515],
                                         tb_t[:, F:F + 515])
                    nc.vector.tensor_add(ya[:, 516:1 + F], ta_t[:, 515:F],
                                         ta_t[:, F + 515:2 * F])
                    nc.vector.tensor_add(yb[:, 516:1 + F], tb_t[:, 515:F],
                                         tb_t[:, F + 515:2 * F])

                    pss = [ps6.tile([64, 512], f32, tag="ps6",
                                    name=f"ps6_{o}_{pt}") for pt in range(2)]
                    for j in range(4):
                        for (icm, ysrc, pk) in ((ica_sb, ya, 0), (icb_sb, yb, 1)):
                            for pt in range(2):
                                c0 = pt * 512 + 3 - j
                                mm(pss[pt][:], icm[:, j * 64:(j + 1) * 64],
                                   ysrc[:, c0:c0 + 512],
                                   start=(j == 0 and pk == 0),
                                   stop=(j == 3 and pk == 1),
                                   reuse=(pt > 0))
                    for pt in range(2):
                        ps = pss[pt]
                        base = o * 1024 + pt * 512
                        if pt == 0:
                            bulk = (yt_sb[:, base + 1:base + 512], ps[:, 1:512])
                            edge = (yt_sb[:, base:base + 1], ps[:, 0:1],
                                    inv3_sb[:, 1:2])
                        else:
                            bulk = (yt_sb[:, base:base + 511], ps[:, 0:511])
                            edge = (yt_sb[:, base + 511:base + 512],
                                    ps[:, 511:512], inv3_sb[:, 2:3])
                        if (o + pt) % 2 == 0:
                            nc.scalar.activation(bulk[0], bulk[1], AF.Identity,
                                                 bias=bias_sb[:, o:o + 1],
                                                 scale=inv3_sb[:, 0:1])
                        else:
                            nc.vector.tensor_scalar(bulk[0], bulk[1],
                                                    inv3_sb[:, 0:1],
                                                    bias_sb[:, o:o + 1],
                                                    ALU.mult, ALU.add)
                        nc.vector.tensor_scalar(edge[0], edge[1], edge[2],
                                                bias_sb[:, o:o + 1],
                                                ALU.mult, ALU.add)
                    nc.sync.dma_start(yt_out[:, o * 1024:(o + 1) * 1024],
                                      yt_sb[:, o * 1024:(o + 1) * 1024])

    nc.compile()
    return nc


def _prep_inputs(x, conditioning, w1, b1, w2, b2, bias):
    c = _consts()
    x = np.asarray(x, dtype=np.float32)
    conditioning = np.asarray(conditioning, dtype=np.float32)
    w1 = np.asarray(w1, dtype=np.float32)
    b1 = np.asarray(b1, dtype=np.float32)
    w2 = np.asarray(w2, dtype=np.float32)
    b2 = np.asarray(b2, dtype=np.float32)
    bias = np.asarray(bias, dtype=np.float32)

    w1t = w1.reshape(32, 32, 4, 64).transpose(1, 3, 2, 0).reshape(2048, 4, 32)
    w1n = np.ascontiguousarray(
        w1t.reshape(16, 128, 128).transpose(1, 0, 2).reshape(128, 2048)
    ).astype(np.float16)
    w2t = w2[:, :, 0].T                                               # [32, 64]
    w2r = np.concatenate([np.tile(w2t[:, 0:32], (1, 4)),
                          np.tile(w2t[:, 32:64], (1, 4))],
                         axis=1).astype(np.float16)                   # [32, 256]
    bias64 = np.tile(bias.reshape(1, 4), (64, 1)).astype(np.float32)
    blk = np.arange(128) // 32
    s1 = (blk % 2 == 1).astype(np.float32)
    s2 = (blk >= 2).astype(np.float32)
    mk = np.stack([s1, 1.0 - s1, s2, 1.0 - s2], axis=1)               # [128, 4]

    shared = {
        "eye_in": np.eye(128, dtype=np.float16),
        "w1n_in": w1n, "w2r_in": w2r, "ws_in": c["WS"],
        "wx_in": c["wx_h"], "ica_in": c["ica"], "icb_in": c["icb"],
        "b1_in": b1.reshape(32, 1).copy(),
        "b1s_in": (0.01 * b1).reshape(32, 1).copy(),
        "b2a_in": np.tile(b2[:32], 4).reshape(128, 1).astype(np.float32),
        "b2w_in": np.tile(b2[32:], 4).reshape(128, 1).astype(np.float32),
        "mk_in": mk, "inv3_in": c["inv3"], "bias_in": bias64,
    }
    in_maps = []
    for b in range(B):
        condpad = np.zeros((CI, T + K), dtype=np.float32)
        condpad[:, 128:128 + T] = conditioning[b]
        d = condpad.reshape(CI, H, 64).transpose(0, 2, 1).reshape(2048, H)
        d = np.ascontiguousarray(d.reshape(16, 128, H)).astype(np.float16)
        xp = np.pad(x[b], ((0, 0), (128, 128)), mode="reflect")
        xd = np.ascontiguousarray(
            xp.reshape(2, H, 64).transpose(0, 2, 1).reshape(2, 64, H)
            .transpose(1, 0, 2).reshape(64, 2 * H)).astype(np.float16)
        m = dict(shared)
        m["d_in"] = d
        m["xd_in"] = xd
        in_maps.append(m)
    return in_maps


def _assemble(results):
    y = np.empty((B, O, T), dtype=np.float32)
    for b in range(B):
        yt = results[b]["yt_out"]                        # [64, 4096]
        y[b] = yt.reshape(64, 4, 1024).transpose(1, 2, 0).reshape(4, T)
    return y


def kernel(x, conditioning, w1, b1, w2, b2, bias):
    from concourse.bass_utils import run_bass_kernel_spmd
    if "nc" not in _prog_cache:
        _prog_cache["nc"] = _build_program()
    nc = _prog_cache["nc"]
    in_maps = _prep_inputs(x, conditioning, w1, b1, w2, b2, bias)
    res = run_bass_kernel_spmd(nc, in_maps, core_ids=list(range(B)))
    return _assemble(res.results)


# revision 41
# speedup vs baseline: 1.1130x; 1.1130x over previous
"""DynamicSincConv1d Trainium2 kernel (v2).

Data-parallel over batch: 8 batch elements -> 8 NeuronCores. All heavy
math runs on-device; the host reshapes inputs into DMA-friendly layouts
and reassembles the output.

Key algorithmic moves vs a direct translation:
 - The windowed-sinc bank is symmetric about k=128, so every filter
   spectrum is real: the complex multiply becomes a real scaling of X
   (fbpack layout: pack a = Re fb 0..127, pack b = [fb128, Im fb 1..127]).
 - sinc(w*t) is approximated by a degree-3 polynomial in (w*t)^2
   (max err 2e-6 over the reachable |w|<=1, |t|<=0.5 range).  The
   filter synthesis + rFFT then factorizes through 4 moments
   P_j = amp*w^(2j): R = WS @ Pm with host-baked WS.  This removes the
   per-(o,i,s) sinc/sin/reciprocal elementwise work and two matmul
   layers entirely.
 - Stage-1 conv uses m=128 output tiles (4 kernel-shifts x 32 channels)
   and a cheap cross-partition j-sum, 4x fewer matmul rows.
 - iSTFT overlap-add folds into PSUM accumulation with shifted rhs
   slices; the window-square normalization is periodic except at the
   two boundary hops, so it reduces to a per-partition tensor_scalar.
 - Matmuls run in fp16/fp32r (1 cycle/row vs 4 for fp32); elementwise
   stages run in fp16 (2x DVE throughput).
"""

import math
import numpy as np

B, CI, I, O, S = 8, 32, 2, 4, 4
K, HOP, T = 256, 64, 65536
F = T // HOP + 1            # 1025 frames
H = (T + K) // HOP          # 1028
PI = math.pi
FT_F = [(0, 512), (512, 512), (1024, 1)]     # tiles of F
FT_H = [(0, 512), (512, 512), (1024, 4)]     # tiles of H

_prog_cache = {}


def _consts():
    n = np.arange(K, dtype=np.float64)
    ola = 0.5 * (1.0 - np.cos(2.0 * PI * n / K))
    fir = 0.42 - 0.5 * np.cos(2.0 * PI * n / K) + 0.08 * np.cos(4.0 * PI * n / K)

    d = np.arange(128, dtype=np.float64)
    fb = np.arange(K // 2 + 1, dtype=np.float64)
    cd = np.where(d == 0, 1.0, 2.0)
    m1full = (((-1.0) ** fb)[None, :] * cd[:, None] * fir[128 + d.astype(int)][:, None]
              / (S * K) * np.cos(2.0 * PI * np.outer(d, fb) / K))     # [128, 129]

    # sinc_n(u) ~= sum_j c_j u^(2j) on u in [0, 0.503]
    us = np.linspace(0, 0.503, 6001)
    V = np.vander(us * us, 4, increasing=True)
    c, *_ = np.linalg.lstsq(V, np.sinc(us), rcond=None)

    pw = (d[:, None] / K) ** (2 * np.arange(4)[None, :])              # [128, 4]
    M2full = np.einsum("df,dj->fj", m1full, pw) * c[None, :]          # [129, 4]
    M2a = M2full[0:128]
    M2b = np.concatenate([M2full[128:129], M2full[1:128]], axis=0)    # [128, 4]

    WS = np.zeros((128, 16, 128), dtype=np.float64)
    for oi in range(8):
        for j in range(4):
            for s in range(4):
                WS[32 * j + 4 * oi + s, oi * 2 + 0, :] = M2a[:, j]
                WS[32 * j + 4 * oi + s, oi * 2 + 1, :] = M2b[:, j]
    WS = np.ascontiguousarray(WS.reshape(128, 2048)).astype(np.float16)

    kk = np.arange(K, dtype=np.float64)
    ang = 2.0 * PI * np.outer(kk, fb) / K
    wre = ola[:, None] * np.cos(ang)
    wim = -ola[:, None] * np.sin(ang)
    colsA = wre[:, 0:128]
    colsB = np.concatenate([wre[:, 128:129], wim[:, 1:128]], axis=1)
    wx_full = np.concatenate([colsA, colsB], axis=1).reshape(4, 64, 256)
    wx_h = np.ascontiguousarray(
        wx_full.transpose(1, 0, 2).reshape(64, 1024)).astype(np.float16)

    cp = np.where(fb == 0, 1.0, 2.0)
    icre = (cp[:, None] / K) * np.cos(2.0 * PI * np.outer(fb, n) / K) * ola[None, :]
    icim = (-2.0 / K) * np.sin(2.0 * PI * np.outer(fb, n) / K) * ola[None, :]
    ica = np.ascontiguousarray(icre[0:128]).astype(np.float16)        # [128, 256]
    icb = np.concatenate([icre[128:129], icim[1:128]], axis=0).astype(np.float16)

    ola2 = ola * ola
    r = np.arange(64)
    env_int = sum(ola2[64 * j + r] for j in range(4))
    env_p0 = sum(ola2[64 * j + r] for j in (0, 1, 2))
    env_p1023 = sum(ola2[64 * j + r] for j in (1, 2, 3))
    inv3 = np.stack([1.0 / env_int, 1.0 / env_p0, 1.0 / env_p1023],
                    axis=1).astype(np.float32)                        # [64, 3]

    return dict(WS=WS, wx_h=wx_h, ica=ica, icb=icb, inv3=inv3)


def _build_program():
    import concourse.bacc as bacc
    import concourse.mybir as mybir
    import concourse.tile as tile

    f32 = mybir.dt.float32
    f16 = mybir.dt.float16
    f32r = mybir.dt.float32r
    AF = mybir.ActivationFunctionType
    ALU = mybir.AluOpType

    nc = bacc.Bacc("TRN2", target_bir_lowering=False, debug=False, num_devices=8)

    eye_in = nc.dram_tensor("eye_in", [128, 128], f16, kind="ExternalInput")
    d_in = nc.dram_tensor("d_in", [16, 128, H], f16, kind="ExternalInput")
    xd_in = nc.dram_tensor("xd_in", [64, 2 * H], f16, kind="ExternalInput")
    w1n_in = nc.dram_tensor("w1n_in", [128, 2048], f16, kind="ExternalInput")
    w2r_in = nc.dram_tensor("w2r_in", [32, 256], f16, kind="ExternalInput")
    ws_in = nc.dram_tensor("ws_in", [128, 2048], f16, kind="ExternalInput")
    wx_in = nc.dram_tensor("wx_in", [64, 1024], f16, kind="ExternalInput")
    ica_in = nc.dram_tensor("ica_in", [128, 256], f16, kind="ExternalInput")
    icb_in = nc.dram_tensor("icb_in", [128, 256], f16, kind="ExternalInput")
    b1_in = nc.dram_tensor("b1_in", [32, 1], f32, kind="ExternalInput")
    b1s_in = nc.dram_tensor("b1s_in", [32, 1], f32, kind="ExternalInput")
    b2a_in = nc.dram_tensor("b2a_in", [128, 1], f32, kind="ExternalInput")
    b2w_in = nc.dram_tensor("b2w_in", [128, 1], f32, kind="ExternalInput")
    mk_in = nc.dram_tensor("mk_in", [128, 4], f32, kind="ExternalInput")
    inv3_in = nc.dram_tensor("inv3_in", [64, 3], f32, kind="ExternalInput")
    bias_in = nc.dram_tensor("bias_in", [64, 4], f32, kind="ExternalInput")
    yt_out = nc.dram_tensor("yt_out", [64, 4096], f32, kind="ExternalOutput")

    with tile.TileContext(nc) as tc:
        with tc.tile_pool(name="cpool", bufs=1) as cpool:
            w1n_sb = cpool.tile([128, 2048], f16, tag="w1n")
            w2r_sb = cpool.tile([32, 256], f16, tag="w2r")
            ws_sb = cpool.tile([128, 2048], f16, tag="ws")
            wx_sb = cpool.tile([64, 1024], f16, tag="wx")
            ica_sb = cpool.tile([128, 256], f16, tag="ica")
            icb_sb = cpool.tile([128, 256], f16, tag="icb")
            b1_sb = cpool.tile([32, 1], f32, tag="b1")
            b1s_sb = cpool.tile([32, 1], f32, tag="b1s")
            b2a_sb = cpool.tile([128, 1], f32, tag="b2a")
            b2w_sb = cpool.tile([128, 1], f32, tag="b2w")
            mk_sb = cpool.tile([128, 4], f32, tag="mk")
            inv3_sb = cpool.tile([64, 3], f32, tag="inv3")
            bias_sb = cpool.tile([64, 4], f32, tag="bias")
            xd_sb = cpool.tile([64, 2 * H], f16, tag="xd")
            xa_sb = cpool.tile([128, 2 * F], f16, tag="xa")
            xb_sb = cpool.tile([128, 2 * F], f16, tag="xb")
            eye_sb = cpool.tile([128, 128], f16, tag="eye")
            wid4_sb = cpool.tile([128, F], f16, tag="wid4")
            wsq4_sb = cpool.tile([128, F], f16, tag="wsq4")
            mb1_sb = cpool.tile([128, F], f16, tag="mb1")
            mb2_sb = cpool.tile([128, F], f16, tag="mb2")
            b2sq_sb = cpool.tile([128, F], f16, tag="b2sq")
            tm_sb = cpool.tile([128, F], f16, tag="tm")
            pm_sb = cpool.tile([128, F], f16, tag="pm")
            outs_sb = cpool.tile([128, H], f16, tag="outs")
            hb_sb = cpool.tile([32, F], f32, tag="hb")
            lt_sb = cpool.tile([32, F], f32, tag="lt")
            h_sb = cpool.tile([32, F], f16, tag="h")
            fpa_sb = cpool.tile([128, 8 * F], f16, tag="fpa")
            fpb_sb = cpool.tile([128, 8 * F], f16, tag="fpb")
            yt_sb = cpool.tile([64, 4096], f32, tag="yt")

            # DMA spread over the two HW DGE queues (sync + scalar).
            # STFT inputs first (gate PE start), then weights, then the
            # big conditioning stream interleaved with small consts.
            with tc.tile_pool(name="dpool", bufs=1) as dpool:
                dts = [dpool.tile([128, H], f16, tag=f"d{c}", name=f"d{c}")
                       for c in range(16)]
                nc.sync.dma_start(wx_sb[:], wx_in[:])
                nc.sync.dma_start(xd_sb[:, 0:H], xd_in[:, 0:H])
                nc.scalar.dma_start(xd_sb[:, H:2 * H], xd_in[:, H:2 * H])
                nc.scalar.dma_start(w1n_sb[:], w1n_in[:])
                nc.scalar.dma_start(ws_sb[:], ws_in[:])
                nc.scalar.dma_start(eye_sb[:], eye_in[:])
                sync_small = [(b1_sb, b1_in), (b1s_sb, b1s_in),
                              (w2r_sb, w2r_in), (b2a_sb, b2a_in),
                              (b2w_sb, b2w_in), (mk_sb, mk_in),
                              (inv3_sb, inv3_in), (bias_sb, bias_in),
                              (ica_sb, ica_in), (icb_sb, icb_in)]
                for c in range(16):
                    eng = nc.sync if c % 2 == 0 else nc.scalar
                    eng.dma_start(dts[c][:], d_in[c])
                    if c % 2 == 0 and sync_small:
                        t_sb, t_in = sync_small.pop(0)
                        nc.sync.dma_start(t_sb[:], t_in[:])
                for t_sb, t_in in sync_small:
                    nc.sync.dma_start(t_sb[:], t_in[:])

                def mm(out, lhsT, rhs, start, stop, reuse=False):
                    """matmul; reuse=True elides the stationary-weight
                    reload (previous matmul loaded the same lhsT)."""
                    inst = nc.tensor.matmul(out, lhsT, rhs,
                                            start=start, stop=stop)
                    if reuse:
                        inst.ins.ldweights = False
                    return inst

                # STFT quarters double as PE filler between dependent stages
                with tc.tile_pool(name="ps4", bufs=1, space="PSUM") as ps4:
                    def stft_quarter(i, mt, xdst):
                        ps = ps4.tile([128, F], f32, tag="ps4", name=f"ps4_{i}{mt}")
                        for j in range(4):
                            for ft, (f0, nf) in enumerate(FT_F):
                                mm(ps[:, f0:f0 + nf],
                                   wx_sb[:, j * 256 + mt * 128:
                                         j * 256 + mt * 128 + 128],
                                   xd_sb[:, i * H + f0 + j:i * H + f0 + j + nf],
                                   start=(j == 0), stop=(j == 3),
                                   reuse=(ft > 0))
                        nc.scalar.activation(xdst[:, i * F:(i + 1) * F],
                                             ps[:], AF.Copy)

                    stft_quarter(0, 0, xa_sb)

                    # ---- stage 1: conditioning conv, m=128 (4j x 32ch) ----
                    with tc.tile_pool(name="ps1", bufs=1, space="PSUM") as ps1:
                        out_ps = ps1.tile([128, H], f32, tag="out1")
                        for c in range(16):
                            for ft, (f0, nf) in enumerate(FT_H):
                                mm(out_ps[:, f0:f0 + nf],
                                   w1n_sb[:, c * 128:(c + 1) * 128],
                                   dts[c][:, f0:f0 + nf],
                                   start=(c == 0), stop=(c == 15),
                                   reuse=(ft > 0))
                        nc.scalar.activation(outs_sb[:, 0:516],
                                             out_ps[:, 0:516], AF.Copy)
                        nc.scalar.activation(outs_sb[:, 516:H],
                                             out_ps[:, 516:H], AF.Copy)

                    stft_quarter(0, 1, xb_sb)

                    # j-sum via 4 shifted accumulating matmuls (identity
                    # lhsT), then bias + leaky_relu
                    with tc.tile_pool(name="psh", bufs=1, space="PSUM") as psh:
                        h_ps = psh.tile([32, F], f32, tag="hps")
                        for j in range(4):
                            for ft, (f0, nf) in enumerate(FT_F):
                                mm(h_ps[:, f0:f0 + nf],
                                   eye_sb[:, j * 32:(j + 1) * 32],
                                   outs_sb[:, f0 + j:f0 + j + nf],
                                   start=(j == 0), stop=(j == 3),
                                   reuse=(ft > 0))
                        nc.scalar.activation(hb_sb[:], h_ps[:], AF.Identity,
                                             bias=b1_sb[:, 0:1])
                        nc.scalar.activation(lt_sb[:], h_ps[:], AF.Identity,
                                             bias=b1s_sb[:, 0:1], scale=0.01)
                        nc.vector.tensor_max(h_sb[:], hb_sb[:], lt_sb[:])

                    stft_quarter(1, 0, xa_sb)

                    # ---- stage 2: 1x1 conv (outputs replicated 4x over
                    # partition blocks) + tanh -> amp4 (in pm) / wid4 ----
                    with tc.tile_pool(name="ps2", bufs=2, space="PSUM") as ps2:
                        for (k0, dst, bcol) in ((0, pm_sb, b2a_sb),
                                                (128, wid4_sb, b2w_sb)):
                            for ft, (f0, nf) in enumerate(FT_F):
                                pa = ps2.tile([128, nf], f32, tag="ps2a")
                                mm(pa[:], w2r_sb[:, k0:k0 + 128],
                                   h_sb[:, f0:f0 + nf],
                                   start=True, stop=True, reuse=(ft > 0))
                                nc.scalar.activation(dst[:, f0:f0 + nf], pa[:],
                                                     AF.Tanh, bias=bcol[:, 0:1])

                    # ---- moments in place: pm = amp * [1,w2,w4,w6] by
                    # partition block via per-partition mask scalars:
                    # mb1 = [1,w2,1,w2], mb2 = [1,1,w2,w2], pm = amp*mb1*mb2^2
                    # Split into column halves so stage-3's first f-tile can
                    # start while the back half is still being computed.
                    for sl in (slice(0, 512), slice(512, F)):
                        nc.vector.tensor_mul(wsq4_sb[:, sl],
                                             wid4_sb[:, sl], wid4_sb[:, sl])
                        nc.vector.tensor_scalar(mb1_sb[:, sl], wsq4_sb[:, sl],
                                                mk_sb[:, 0:1], mk_sb[:, 1:2],
                                                ALU.mult, ALU.add)
                        nc.vector.tensor_scalar(mb2_sb[:, sl], wsq4_sb[:, sl],
                                                mk_sb[:, 2:3], mk_sb[:, 3:4],
                                                ALU.mult, ALU.add)
                        nc.vector.tensor_mul(b2sq_sb[:, sl],
                                             mb2_sb[:, sl], mb2_sb[:, sl])
                        nc.vector.tensor_mul(tm_sb[:, sl],
                                             pm_sb[:, sl], mb1_sb[:, sl])
                        nc.vector.tensor_mul(pm_sb[:, sl],
                                             tm_sb[:, sl], b2sq_sb[:, sl])

                    stft_quarter(1, 1, xb_sb)

            # ---- stage 3: R = WS @ Pm per (oi, pack) -> fpa/fpb f16 ----
            with tc.tile_pool(name="ps3", bufs=2, space="PSUM") as ps3:
                for oi in range(8):
                    for (p, dest) in ((0, fpa_sb), (1, fpb_sb)):
                        ps = ps3.tile([128, F], f32, tag="ps3")
                        for ft, (f0, nf) in enumerate(FT_F):
                            mm(ps[:, f0:f0 + nf],
                               ws_sb[:, (oi * 2 + p) * 128:
                                     (oi * 2 + p + 1) * 128],
                               pm_sb[:, f0:f0 + nf],
                               start=True, stop=True, reuse=(ft > 0))
                        dst = dest[:, oi * F:(oi + 1) * F]
                        if p == 0 or oi % 2 == 0:
                            nc.scalar.activation(dst, ps[:], AF.Copy)
                        else:
                            nc.vector.tensor_scalar(dst, ps[:], 1.0,
                                                    None, ALU.mult)

            # ---- stage 5+6 per o: cmul, iSTFT with OLA in PSUM ----
            with tc.tile_pool(name="yp", bufs=2) as ypool, \
                 tc.tile_pool(name="ctp", bufs=2) as ctpool, \
                 tc.tile_pool(name="ps6", bufs=4, space="PSUM") as ps6:
                for o in range(4):
                    ya = ypool.tile([128, H], f16, tag="ya")
                    yb = ypool.tile([128, H], f16, tag="yb")
                    ta_t = ctpool.tile([128, 2 * F], f16, tag="cta")
                    tb_t = ctpool.tile([128, 2 * F], f16, tag="ctb")
                    o2 = 2 * o
                    nc.gpsimd.memset(ya[:, 0:1], 0.0)
                    nc.gpsimd.memset(ya[:, 1026:1028], 0.0)
                    nc.gpsimd.memset(yb[:, 0:1], 0.0)
                    nc.gpsimd.memset(yb[:, 1026:1028], 0.0)
                    # all on DVE (gpsimd is ~2-3x slower on wide ops); adds
                    # split at col 515 so the pt=0 iSTFT matmuls can start
                    # before the back half of ya/yb is summed
                    nc.vector.tensor_mul(ta_t[:], xa_sb[:],
                                         fpa_sb[:, o2 * F:(o2 + 2) * F])
                    nc.vector.tensor_mul(tb_t[:], xb_sb[:],
                                         fpb_sb[:, o2 * F:(o2 + 2) * F])
                    nc.vector.tensor_add(ya[:, 1:516], ta_t[:, 0:515],
                                         ta_t[:, F:F + 515])
                    nc.vector.tensor_add(yb[:, 1:516], tb_t[:, 0:515],
                                         tb_t[:, F:F + 515])
                    nc.vector.tensor_add(ya[:, 516:1 + F], ta_t[:, 515:F],
                                         ta_t[:, F + 515:2 * F])
                    nc.vector.tensor_add(yb[:, 516:1 + F], tb_t[:, 515:F],
                                         tb_t[:, F + 515:2 * F])

                    pss = [ps6.tile([64, 512], f32, tag="ps6",
                                    name=f"ps6_{o}_{pt}") for pt in range(2)]
                    for j in range(4):
                        for (icm, ysrc, pk) in ((ica_sb, ya, 0), (icb_sb, yb, 1)):
                            for pt in range(2):
                                c0 = pt * 512 + 3 - j
                                mm(pss[pt][:], icm[:, j * 64:(j + 1) * 64],
                                   ysrc[:, c0:c0 + 512],
                                   start=(j == 0 and pk == 0),
                                   stop=(j == 3 and pk == 1),
                                   reuse=(pt > 0))
                    for pt in range(2):
                        ps = pss[pt]
                        base = o * 1024 + pt * 512
                        if pt == 0:
                            bulk = (yt_sb[:, base + 1:base + 512], ps[:, 1:512])
                            edge = (yt_sb[:, base:base + 1], ps[:, 0:1],
                                    inv3_sb[:, 1:2])
                        else:
                            bulk = (yt_sb[:, base:base + 511], ps[:, 0:511])
                            edge = (yt_sb[:, base + 511:base + 512],
                                    ps[:, 511:512], inv3_sb[:, 2:3])
                        if (o + pt) % 2 == 0:
                            nc.scalar.activation(bulk[0], bulk[1], AF.Identity,
                                                 bias=bias_sb[:, o:o + 1],
                                                 scale=inv3_sb[:, 0:1])
                        else:
                            nc.vector.tensor_scalar(bulk[0], bulk[1],
                                                    inv3_sb[:, 0:1],
                                                    bias_sb[:, o:o + 1],
                                                    ALU.mult, ALU.add)
                        nc.vector.tensor_scalar(edge[0], edge[1], edge[2],
                                                bias_sb[:, o:o + 1],
                                                ALU.mult, ALU.add)
                    nc.sync.dma_start(yt_out[:, o * 1024:(o + 1) * 1024],
                                      yt_sb[:, o * 1024:(o + 1) * 1024])

    nc.compile()
    return nc


def _prep_inputs(x, conditioning, w1, b1, w2, b2, bias):
    c = _consts()
    x = np.asarray(x, dtype=np.float32)
    conditioning = np.asarray(conditioning, dtype=np.float32)
    w1 = np.asarray(w1, dtype=np.float32)
    b1 = np.asarray(b1, dtype=np.float32)
    w2 = np.asarray(w2, dtype=np.float32)
    b2 = np.asarray(b2, dtype=np.float32)
    bias = np.asarray(bias, dtype=np.float32)

    w1t = w1.reshape(32, 32, 4, 64).transpose(1, 3, 2, 0).reshape(2048, 4, 32)
    w1n = np.ascontiguousarray(
        w1t.reshape(16, 128, 128).transpose(1, 0, 2).reshape(128, 2048)
    ).astype(np.float16)
    w2t = w2[:, :, 0].T                                               # [32, 64]
    w2r = np.concatenate([np.tile(w2t[:, 0:32], (1, 4)),
                          np.tile(w2t[:, 32:64], (1, 4))],
                         axis=1).astype(np.float16)                   # [32, 256]
    bias64 = np.tile(bias.reshape(1, 4), (64, 1)).astype(np.float32)
    blk = np.arange(128) // 32
    s1 = (blk % 2 == 1).astype(np.float32)
    s2 = (blk >= 2).astype(np.float32)
    mk = np.stack([s1, 1.0 - s1, s2, 1.0 - s2], axis=1)               # [128, 4]

    shared = {
        "eye_in": np.eye(128, dtype=np.float16),
        "w1n_in": w1n, "w2r_in": w2r, "ws_in": c["WS"],
        "wx_in": c["wx_h"], "ica_in": c["ica"], "icb_in": c["icb"],
        "b1_in": b1.reshape(32, 1).copy(),
        "b1s_in": (0.01 * b1).reshape(32, 1).copy(),
        "b2a_in": np.tile(b2[:32], 4).reshape(128, 1).astype(np.float32),
        "b2w_in": np.tile(b2[32:], 4).reshape(128, 1).astype(np.float32),
        "mk_in": mk, "inv3_in": c["inv3"], "bias_in": bias64,
    }
    in_maps = []
    for b in range(B):
        condpad = np.zeros((CI, T + K), dtype=np.float32)
        condpad[:, 128:128 + T] = conditioning[b]
        d = condpad.reshape(CI, H, 64).transpose(0, 2, 1).reshape(2048, H)
        d = np.ascontiguousarray(d.reshape(16, 128, H)).astype(np.float16)
        xp = np.pad(x[b], ((0, 0), (128, 128)), mode="reflect")
        xd = np.ascontiguousarray(
            xp.reshape(2, H, 64).transpose(0, 2, 1).reshape(2, 64, H)
            .transpose(1, 0, 2).reshape(64, 2 * H)).astype(np.float16)
        m = dict(shared)
        m["d_in"] = d
        m["xd_in"] = xd
        in_maps.append(m)
    return in_maps


def _assemble(results):
    y = np.empty((B, O, T), dtype=np.float32)
    for b in range(B):
        yt = results[b]["yt_out"]                        # [64, 4096]
        y[b] = yt.reshape(64, 4, 1024).transpose(1, 2, 0).reshape(4, T)
    return y


def kernel(x, conditioning, w1, b1, w2, b2, bias):
    from concourse.bass_utils import run_bass_kernel_spmd
    if "nc" not in _prog_cache:
        _prog_cache["nc"] = _build_program()
    nc = _prog_cache["nc"]
    in_maps = _prep_inputs(x, conditioning, w1, b1, w2, b2, bias)
    res = run_bass_kernel_spmd(nc, in_maps, core_ids=list(range(B)))
    return _assemble(res.results)
